# revision 27
# baseline (speedup 1.0000x reference)
"""GCN encoder (2x GCNConv + BatchNorm + ReLU) on 8 Trainium2 NeuronCores.

Strategy (graph/data parallel, per sharding hint):
- Nodes are permuted (degree-sorted, round-robin dealt) and sharded across the
  8 cores; each core owns 49 "windows" of 128 destination nodes.
- conv1 aggregates FIRST, transforms SECOND: out1 = ((D^-.5 A D^-.5) x) @ W1.
  The gathered table is dis*x quantized to fp8 e3m4 (512 B rows, so the DMA
  cost model's <512B 2x latency multiplier doesn't apply), pre-scaled by a
  power-of-two s for fp8 range; s is divided back out via the dis[d]
  diagonal.  There is no per-core x@W1 phase at all - the host ships the fp8
  table directly, saving ~215 us of xb-load/h1-write DMA and ~170 us of
  redundant PE time per core versus materializing h1 = x@W1 on device.
- Aggregation happens TRANSPOSED: for each gathered chunk (128 messages) and
  each 128-feature group, one matmul chunk_fg^T @ diag(dis[d]/s) accumulates
  into feature-major PSUM.  This lands the window aggregate directly in the
  layout BatchNorm/W1 need (no transpose stage, no slot-major eviction) and
  folds the dis[d] scaling in for free.  The per-bank pairs of accumulation
  groups rely on PSUM pending-zero semantics (start=True only on the first
  group member per bank).
- Windows are processed in an interleaved light/heavy order (they are
  degree-sorted, so the head windows have ~65 chunks and the tail ~20): a
  PE-bound heavy window always has a DMA-bound light neighbor in flight.
  The gather index stream is laid out in this processing order; the last
  processed windows are light so the pipeline drains quickly.
- o1T = W1^T @ aggT is computed one window behind the aggregation (software
  pipelining); BN1 statistics are free-dim DVE reductions of o1T, with the
  partial sums over the first 42 processed windows shipped early so the
  cross-core stats AllGather ([128,8] + on-chip sum) fires right at conv1
  end.  BN2 stats are split the same way (PSUM groups A/B).
- Self loops never enter the gather stream: conv1's self message is the
  core's own fp8 table row (xown, resident in SBUF), added as one extra
  matmul per window; conv2's is the core's own h2stage row.
- conv1 and conv2 share ONE int16 index stream: both tables use the same
  [8 x (SLOTS+1)] row layout (trailing zero row per core block) and both
  convs aggregate the same real-edge set.  The stream is split in lo/hi
  table halves for int16 range.
- h2 = relu(bn1(o1)) @ W2 (fp16) is computed per-shard and replicated via
  AllGather; conv2 then gathers 256 B fp16 rows and aggregates slot-major
  with identity matmuls (fp8 does not help conv2: dma_gather's elem size
  floor is 256 B and sub-512 B rows pay the 2x multiplier either way).
- b1/b2 are ignored: a per-feature constant added before BatchNorm cancels
  exactly in (x - mean).
"""

import sys

sys.path.insert(0, "/opt/trn_rl_repo")

import numpy as np

N_CORES = 8
P = 128
EPS = 1e-5

_FULL_CFG = dict(N=50000, IN=512, D1=256, D2=128)


# ---------------------------------------------------------------- host preprocessing

def _preprocess(edge_index, N):
    """Graph preprocessing: node permutation, slot/chunk assignment, gather
    indices.  Pure integer work on the host."""
    src = np.asarray(edge_index[0], dtype=np.int64)
    dst = np.asarray(edge_index[1], dtype=np.int64)
    loop = np.arange(N, dtype=np.int64)
    D_all = np.concatenate([dst, loop])

    deg = np.bincount(D_all, minlength=N)  # >= 1 (self loop)
    dis = (1.0 / np.sqrt(deg.astype(np.float64))).astype(np.float32)

    real_pc = N // N_CORES
    WPC = (real_pc + P - 1) // P          # windows per core
    SLOTS = WPC * P                        # slot positions per core
    BLK = SLOTS + 1                        # +1 trailing zero row per core block

    # deal nodes to cores round-robin in degree-desc order
    order = np.argsort(-deg, kind="stable")
    core_of = np.empty(N, dtype=np.int64)
    core_of[order] = np.arange(N) % N_CORES

    half_node = core_of >= (N_CORES // 2)  # False = lo table half
    halfE = half_node[src]                 # real edges only

    deg_lo = np.bincount(dst[~halfE], minlength=N)
    deg_hi = np.bincount(dst[halfE], minlength=N)

    # position of each node within its core: sort by (deg_lo desc, deg_hi
    # desc), then re-sort blocks by deg_hi - keeps per-window max(deg_lo) and
    # max(deg_hi) both tight
    pos = np.empty(N, dtype=np.int64)
    node_by_cp = np.full((N_CORES, SLOTS), -1, dtype=np.int64)
    RESORT_BLK = 896
    for c in range(N_CORES):
        nodes_c = np.flatnonzero(core_of == c)
        o = np.lexsort((-deg_hi[nodes_c], -deg_lo[nodes_c]))
        for s in range(0, len(o), RESORT_BLK):
            blk = o[s : s + RESORT_BLK]
            o[s : s + RESORT_BLK] = blk[np.argsort(-deg_hi[nodes_c][blk], kind="stable")]
        snodes = nodes_c[o]
        pos[snodes] = np.arange(len(snodes))
        node_by_cp[c, : len(snodes)] = snodes

    # per-core per-window chunk counts -> global max (SPMD static shapes)
    dlo_cp = np.zeros((N_CORES, SLOTS), dtype=np.int64)
    dhi_cp = np.zeros((N_CORES, SLOTS), dtype=np.int64)
    m = node_by_cp >= 0
    dlo_cp[m] = deg_lo[node_by_cp[m]]
    dhi_cp[m] = deg_hi[node_by_cp[m]]
    NL = dlo_cp.reshape(N_CORES, WPC, P).max(axis=2).max(axis=0)
    NH = dhi_cp.reshape(N_CORES, WPC, P).max(axis=2).max(axis=0)

    # window processing order: interleave light (tail) and heavy (head)
    # windows so DMA-bound and PE-bound windows overlap in the pipeline
    worder = []
    lo_i, hi_i = WPC - 1, 0
    while lo_i >= hi_i:
        worder.append(lo_i)
        if hi_i < lo_i:
            worder.append(hi_i)
        lo_i -= 1
        hi_i += 1
    # drain the pipeline on light windows: move the 2nd/3rd-lightest (ids
    # WPC-2, WPC-3) to the end of the processing order
    if WPC > 6:
        for wl in (WPC - 3, WPC - 2):
            worder.remove(wl)
            worder.append(wl)
    worder = np.array(worder)

    # gather index stream: segments laid out in processing order, each
    # window's segment is [lo seg][hi seg], chunk-major
    seg = (NL + NH) * P
    base = np.concatenate([[0], np.cumsum(seg[worder])])
    offL = np.empty(WPC, np.int64)
    offL[worder] = base[:-1]
    offH = offL + NL * P
    TOT = int(base[-1])

    # chunk index of each edge among its (dst, half) group
    key = dst * 2 + halfE
    ksort = np.argsort(key, kind="stable")
    skey = key[ksort]
    starts = np.concatenate([[0], np.flatnonzero(np.diff(skey)) + 1])
    group_len = np.diff(np.concatenate([starts, [len(skey)]]))
    chunk_sorted = np.arange(len(skey)) - np.repeat(starts, group_len)
    chunk = np.empty(len(src), dtype=np.int64)
    chunk[ksort] = chunk_sorted

    cD = core_of[dst]
    wD = pos[dst] // P
    slotD = pos[dst] % P
    absrow = core_of[src] * BLK + pos[src]
    rel = np.where(~halfE, absrow, absrow - (N_CORES // 2) * BLK)
    assert rel.max() < 32768
    epos = np.where(~halfE, offL[wD], offH[wD]) + chunk * P + slotD

    PADIDX = SLOTS  # block 0's trailing zero row (rel within half view)
    flat = np.full(N_CORES * TOT, PADIDX, dtype=np.int16)
    flat[cD * TOT + epos] = rel.astype(np.int16)
    flat = flat.reshape(N_CORES, TOT)
    # wrap: idx i -> [i%16, i//16], replicated across the 8 groups of 16
    wrapped16 = flat.reshape(N_CORES, TOT // 16, 16).transpose(0, 2, 1)
    idx_wrapped = np.tile(wrapped16, (1, P // 16, 1))

    # per-core dis (by slot), 1.0 for dummies
    dis_cp = np.ones((N_CORES, SLOTS), dtype=np.float32)
    dis_cp[m] = dis[node_by_cp[m]]
    dismy = dis_cp.reshape(N_CORES, WPC, P).transpose(0, 2, 1)  # [c, 128, WPC]

    # stats mask for conv2 (dummy slots have nonzero h2 = relu(bn1(0)))
    n_dummy = SLOTS - real_pc
    statmask = np.ones((P, 2), dtype=np.float32)
    if n_dummy:
        statmask[P - n_dummy :, 1] = 0.0

    waste = float(TOT) / max(1, len(src) / N_CORES) - 1.0
    return dict(
        WPC=WPC, SLOTS=SLOTS, BLK=BLK, worder=worder,
        NL=NL.astype(int), NH=NH.astype(int), TOT=TOT,
        offL=offL, offH=offH, idx_wrapped=idx_wrapped,
        dismy=dismy, statmask=statmask,
        node_by_cp=node_by_cp, pos=pos, core_of=core_of,
        dis=dis, real_pc=real_pc, waste=waste,
    )


def _pack_inputs(x, W1, W2, gamma1, beta1, pp, cfg):
    """Build the per-core / shared device input arrays."""
    import ml_dtypes

    f16 = np.float16
    f8 = ml_dtypes.float8_e3m4
    N, IN, D1, D2 = cfg["N"], cfg["IN"], cfg["D1"], cfg["D2"]
    WPC, SLOTS, BLK = pp["WPC"], pp["SLOTS"], pp["BLK"]
    KC, KC2 = IN // P, D1 // P
    NROWS = N_CORES * BLK

    # fp8 quantization scale: power of two, headroom below e3m4 max (15.5),
    # large enough to keep the bulk of values out of the denormal zone
    v = np.asarray(x, np.float32) * pp["dis"][:, None]
    maxv = float(np.abs(v).max())
    s = float(2.0 ** np.floor(np.log2(14.0 / maxv))) if maxv > 0 else 1.0

    xtab = np.zeros((NROWS, IN), dtype=f8)
    m = pp["node_by_cp"] >= 0
    rows = (np.arange(N_CORES)[:, None] * BLK + np.arange(SLOTS)[None, :])[m]
    xtab[rows] = (v[pp["node_by_cp"][m]] * s).astype(f8)

    xown = np.empty((N_CORES, P, WPC, IN), dtype=f8)
    for c in range(N_CORES):
        xown[c] = xtab[c * BLK : c * BLK + SLOTS].reshape(WPC, P, IN).transpose(1, 0, 2)

    # dismy2[c, :, 0, :] = dis/s (conv1 eviction), [:, 1, :] = dis (h2 / conv2)
    dismy2 = np.empty((N_CORES, P, 2, WPC), dtype=np.float32)
    dismy2[:, :, 0, :] = pp["dismy"] / s
    dismy2[:, :, 1, :] = pp["dismy"]

    w1b = W1.reshape(KC, P, D1).transpose(1, 0, 2).astype(f16)        # [p, kc, D1]
    w2b = W2.reshape(KC2, P, D2).transpose(1, 0, 2).astype(f16)       # [p, kc, D2]
    g1t = np.asarray(gamma1, np.float32).reshape(KC2, P).T.copy()     # [p, kc2]
    b1t = np.asarray(beta1, np.float32).reshape(KC2, P).T.copy()

    i2 = np.zeros((P, P), dtype=f8)
    i2[np.arange(P), np.arange(P)] = 1.0
    return xtab, xown, dismy2, w1b, w2b, g1t, b1t, i2


# ---------------------------------------------------------------- device kernel

def _build_kernel(cfg, pp, phases=5):
    import concourse.bacc as bacc
    import concourse.mybir as mybir
    import concourse.tile as tile
    from concourse.masks import make_identity
    from contextlib import ExitStack

    N, IN, D1, D2 = cfg["N"], cfg["IN"], cfg["D1"], cfg["D2"]
    WPC, SLOTS, BLK = pp["WPC"], pp["SLOTS"], pp["BLK"]
    NL, NH, TOT = pp["NL"], pp["NH"], pp["TOT"]
    offL, offH = pp["offL"], pp["offH"]
    KC = IN // P
    KC2 = D1 // P
    HB = (N_CORES // 2) * BLK        # hi half base row
    NROWS = N_CORES * BLK            # table rows
    NTmax = int((NL + NH).max())
    RG = [list(range(N_CORES))]
    f32, f16, i16 = mybir.dt.float32, mybir.dt.float16, mybir.dt.int16
    f8 = mybir.dt.float8e3
    AF = mybir.ActivationFunctionType
    AX = mybir.AxisListType
    ALU = mybir.AluOpType

    nc = bacc.Bacc(num_devices=N_CORES)

    # ---- I/O
    xtab_d = nc.dram_tensor("xtab", [NROWS, IN], f8, kind="ExternalInput")
    xown_d = nc.dram_tensor("xown", [P, WPC, IN], f8, kind="ExternalInput")
    idx_d = nc.dram_tensor("idx", [P, TOT // 16], i16, kind="ExternalInput")
    dismy_d = nc.dram_tensor("dismy", [P, 2, WPC], f32, kind="ExternalInput")
    i2_d = nc.dram_tensor("i2", [P, P], f8, kind="ExternalInput")
    w1_d = nc.dram_tensor("w1b", [P, KC, D1], f16, kind="ExternalInput")
    w2_d = nc.dram_tensor("w2b", [P, KC2, D2], f16, kind="ExternalInput")
    g1t_d = nc.dram_tensor("g1t", [P, KC2], f32, kind="ExternalInput")
    b1t_d = nc.dram_tensor("b1t", [P, KC2], f32, kind="ExternalInput")
    mask_d = nc.dram_tensor("statmask", [P, 2], f32, kind="ExternalInput")
    g2_d = nc.dram_tensor("gamma2", [1, D2], f32, kind="ExternalInput")
    b2_d = nc.dram_tensor("beta2", [1, D2], f32, kind="ExternalInput")
    out_d = nc.dram_tensor("out", [SLOTS, D2], f32, kind="ExternalOutput")

    # ---- internal DRAM
    h2shard = nc.dram_tensor("h2shard", [BLK, D2], f16, kind="Internal")
    h2tab = nc.dram_tensor("h2tab", [NROWS, D2], f16, kind="Internal", addr_space="Shared")
    ar1_in = nc.dram_tensor("ar1_in", [P, 8], f32, kind="Internal")
    ar1_out = nc.dram_tensor("ar1_out", [N_CORES, P, 8], f32, kind="Internal", addr_space="Shared")
    ar2_in = nc.dram_tensor("ar2_in", [1, 4 * D2], f32, kind="Internal")
    ar2_out = nc.dram_tensor("ar2_out", [N_CORES, 4 * D2], f32, kind="Internal", addr_space="Shared")

    with tile.TileContext(nc) as tc:
        es = ExitStack()
        with es:
            cpool = es.enter_context(tc.tile_pool(name="const", bufs=1))
            # idx stream first: its load DMA heads the queue, the first
            # gather depends on it
            idx_s = cpool.tile([P, TOT // 16], i16)
            wo = pp["worder"]
            c0 = int(offL[wo[6]]) // 16 if WPC > 6 else TOT // 16
            nc.sync.dma_start(out=idx_s[:, 0:c0], in_=idx_d[:, 0:c0])
            ident_b = cpool.tile([P, P], f16)
            make_identity(nc, ident_b[:])
            i2_s = cpool.tile([P, P], f8)
            nc.scalar.dma_start(out=i2_s[:], in_=i2_d[:, :])
            w1_s = cpool.tile([P, KC, D1], f16)
            nc.scalar.dma_start(out=w1_s[:], in_=w1_d[:, :, :])
            w2_s = cpool.tile([P, KC2, D2], f16)
            nc.scalar.dma_start(out=w2_s[:], in_=w2_d[:, :, :])
            dismy_s = cpool.tile([P, 2, WPC], f32)
            nc.scalar.dma_start(out=dismy_s[:], in_=dismy_d[:, :, :])
            g1t_s = cpool.tile([P, KC2], f32)
            nc.scalar.dma_start(out=g1t_s[:], in_=g1t_d[:, :])
            b1t_s = cpool.tile([P, KC2], f32)
            nc.scalar.dma_start(out=b1t_s[:], in_=b1t_d[:, :])
            mask_s = cpool.tile([P, 2], f32)
            nc.scalar.dma_start(out=mask_s[:], in_=mask_d[:, :])
            mask_b = cpool.tile([P, 2], f16)
            nc.vector.tensor_copy(out=mask_b[:], in_=mask_s[:])
            gb2_s = cpool.tile([1, 2 * D2], f32)  # gamma2|beta2
            nc.scalar.dma_start(out=gb2_s[:, 0:D2], in_=g2_d[:, :])
            nc.scalar.dma_start(out=gb2_s[:, D2:], in_=b2_d[:, :])
            # h2stage doubles as the conv2 self-loop rhs
            h2stage = cpool.tile([P, WPC, D2], f16)

            # zero pad row of h2shard (row SLOTS)
            zrow2 = cpool.tile([1, D2], f16)
            nc.vector.memset(zrow2[:], 0)
            nc.sync.dma_start(out=h2shard[SLOTS:BLK, :], in_=zrow2[:])

            # conv1-lifetime pools
            es1 = ExitStack()
            o1_pool = es1.enter_context(tc.tile_pool(name="o1", bufs=1))
            xown_s = o1_pool.tile([P, WPC, IN], f8)
            # first two processed windows (ids WPC-1 and 0) up front
            nc.sync.dma_start(out=xown_s[:, 0:1], in_=xown_d[:, 0:1, :])
            nc.sync.dma_start(out=xown_s[:, WPC - 1 :], in_=xown_d[:, WPC - 1 :, :])
            o1T = o1_pool.tile([P, WPC, KC2, P], f16)
            stacc = o1_pool.tile([P, WPC, 4], f32)

            # ---------------- conv1: aggregate-first windows ----------------
            lo_view = xtab_d[0:HB, :]
            hi_view = xtab_d[HB:NROWS, :]
            KD, KP = int(__import__('os').environ.get('K_KD', 999)), int(__import__('os').environ.get('K_KP', 0))          # tail chunks summed on DVE / Pool per window
            SPLITW = max(1, WPC - 7)   # pipeline the stats reduction at this window
            with (
                tc.tile_pool(name="g1", bufs=3) as gpool,
                tc.tile_pool(name="ag", bufs=4) as apool,
                tc.tile_pool(name="at", bufs=3) as atpool,
                tc.tile_pool(name="sq1", bufs=4) as sqpool,
                tc.tile_pool(name="acd", bufs=3) as adpool,
                tc.tile_pool(name="c1p", bufs=3, space="PSUM") as wpool,
                tc.tile_pool(name="c1q", bufs=3, space="PSUM") as wpool2c,
                tc.tile_pool(name="o1p", bufs=2, space="PSUM") as o1ppool,
            ):
                aggq, trq = [], []
                # interleaved light/heavy processing order (see _preprocess):
                # a heavy PE-bound window always has a light DMA-bound
                # neighbor in flight, so neither engine starves
                for k, w in enumerate(pp["worder"]):
                    nl, nh = int(NL[w]), int(NH[w])
                    nt = nl + nh
                    gb = gpool.tile([P, NTmax, IN], f8, tag="g1")
                    pieces = 3 if k == 0 else 2 if k <= 2 or k >= WPC - 2 else 1
                    for n0, bs, view, off in ((nl, 0, lo_view, offL[w]),
                                              (nh, nl, hi_view, offH[w])):
                        if not n0:
                            continue
                        cuts = [n0 * i // pieces for i in range(pieces + 1)]
                        for a, b in zip(cuts[:-1], cuts[1:]):
                            if b == a:
                                continue
                            nc.gpsimd.dma_gather(
                                gb[:, bs + a : bs + b, :], view,
                                idx_s[:, off // 16 + a * 8 : off // 16 + b * 8],
                                (b - a) * P, (b - a) * P, IN,
                                elem_step=IN, single_packet=False,
                            )
                    if k == 1:
                        # deferred bulk loads: issued from the Pool stream here
                        # so the first windows' gather transfers aren't queued
                        # behind them on the DMA engines
                        if c0 < TOT // 16:
                            nc.gpsimd.dma_start(out=idx_s[:, c0:], in_=idx_d[:, c0:])
                        nc.gpsimd.dma_start(out=xown_s[:, 1 : WPC - 1], in_=xown_d[:, 1 : WPC - 1, :])
                    assert nt > 0
                    # heavy windows are PE-bound: pre-sum their tail chunks on
                    # the (otherwise idle) DVE into one f16 partial, which then
                    # joins the aggregation as a single extra chunk
                    kd = max(0, min(14, nt - 26))
                    if kd >= 2:
                        accD = adpool.tile([P, IN], f16, tag="accD")
                        nc.vector.tensor_add(accD[:], gb[:, nt - kd, :], gb[:, nt - kd + 1, :])
                        for j in range(nt - kd + 2, nt):
                            nc.vector.tensor_add(accD[:], accD[:], gb[:, j, :])
                    else:
                        kd = 0
                    # diag(dis[d]/s) for this window, used as the moving rhs
                    # of every aggregation matmul: out += chunk_fg^T @ diag
                    # lands the aggregate directly FEATURE-MAJOR and
                    # dis-scaled - no transpose stage, no slot-major evict
                    diag = apool.tile([P, P], f16, tag="diag")
                    nc.vector.tensor_scalar_mul(
                        diag[:], ident_b[:], dismy_s[:, 0, w : w + 1]
                    )
                    # two PSUM tiles of two feature-chunk accumulators each;
                    # the 4 groups per bank-pair rely on pending-zero: only
                    # the first matmul of each bank uses start=True
                    psA = wpool.tile([P, 2, P], f32, tag="psA")
                    psB = wpool2c.tile([P, 2, P], f32, tag="psB")
                    srcs = [gb[:, j, :] for j in range(nt - kd)]
                    srcs.append(xown_s[:, w, :])
                    if kd:
                        srcs.append(accD[:])
                    for j, src_ap in enumerate(srcs):
                        for fg in range(KC):
                            tgt, fh = (psA, fg) if fg < 2 else (psB, fg - 2)
                            nc.tensor.matmul(
                                out=tgt[:, fh, :],
                                lhsT=src_ap[:, fg * P : (fg + 1) * P],
                                rhs=diag[:],
                                start=(j == 0 and fh == 0),
                                stop=(j == len(srcs) - 1 and fh == 1),
                                skip_group_check=True,
                            )
                    aggq.append((w, k, psA, psB))

                    def do_w1(wd, kd2, pA, pB):
                        # evict the transposed aggregate, then one PSUM group
                        # for both W1 halves
                        aT = atpool.tile([P, KC, P], f16, tag="at")
                        nc.scalar.activation(aT[:, 0:2, :], pA[:], AF.Copy)
                        nc.scalar.activation(aT[:, 2:4, :], pB[:], AF.Copy)
                        trq.append((wd, kd2, aT))

                    def do_w1_mm(wd, kd2, aT):
                        op = o1ppool.tile([P, KC2, P], f32, tag="o1p")
                        for h in range(KC2):
                            for kc in range(KC):
                                nc.tensor.matmul(
                                    out=op[:, h, :],
                                    lhsT=w1_s[:, kc, h * P : (h + 1) * P],
                                    rhs=aT[:, kc, :],
                                    start=(h == 0 and kc == 0),
                                    stop=(h == KC2 - 1 and kc == KC - 1),
                                    skip_group_check=True,
                                )
                        nc.scalar.activation(o1T[:, wd, :, :], op[:], AF.Copy)
                        sq = sqpool.tile([P, KC2, P], f16, tag="sq")
                        nc.vector.tensor_mul(sq[:], o1T[:, wd], o1T[:, wd])
                        nc.vector.tensor_reduce(
                            stacc[:, kd2, 0:2], o1T[:, wd], axis=AX.X, op=ALU.add
                        )
                        nc.vector.tensor_reduce(
                            stacc[:, kd2, 2:4], sq[:], axis=AX.X, op=ALU.add
                        )
                        if kd2 == SPLITW - 1:
                            # exact partial sums over the first SPLITW processed
                            # windows, shipped early so the stats AllGather
                            # fires right at conv1 end
                            stA = o1_pool.tile([P, 4], f32)
                            nc.vector.tensor_reduce(
                                stA[:], stacc[:, 0:SPLITW].rearrange("p w f -> p f w"),
                                axis=AX.X, op=ALU.add,
                            )
                            nc.scalar.dma_start(out=ar1_in[:, 0:4], in_=stA[:])

                    def pipeline_step(last):
                        if len(aggq) > (0 if last else 1):
                            do_w1(*aggq.pop(0))
                        if len(trq) > (0 if last else 1):
                            do_w1_mm(*trq.pop(0))

                    pipeline_step(False)
                    if k == WPC - 1:
                        while aggq or trq:
                            pipeline_step(True)
                stB = o1_pool.tile([P, 4], f32)
                nc.vector.tensor_reduce(
                    stB[:], stacc[:, SPLITW:].rearrange("p w f -> p f w"),
                    axis=AX.X, op=ALU.add,
                )
                nc.scalar.dma_start(out=ar1_in[:, 4:8], in_=stB[:])
            if phases < 3:
                outst = cpool.tile([P, WPC, D2], f32)
                nc.vector.memset(outst[:], 0)
                nc.gpsimd.dma_start(
                    out=out_d[0:SLOTS, :].rearrange("(t p) d -> p t d", p=P),
                    in_=outst[:],
                )
                return nc
            nc.gpsimd.collective_compute(
                "AllGather", mybir.AluOpType.bypass,
                ins=[ar1_in[:, :]], outs=[ar1_out[:, :, :]], replica_groups=RG,
            )

            # ---------------- BN1 factors + h2 shard ----------------
            bnp = es1.enter_context(tc.tile_pool(name="bn1", bufs=1))
            sg8 = bnp.tile([P, N_CORES, 8], f32)
            nc.sync.dma_start(out=sg8[:], in_=ar1_out[:, :, :].rearrange("c p f -> p c f"))
            stT8 = bnp.tile([P, 8], f32)
            nc.vector.tensor_reduce(
                stT8[:], sg8[:].rearrange("p c f -> p f c"), axis=AX.X, op=ALU.add
            )
            stTC = bnp.tile([P, 4], f32)
            nc.vector.tensor_add(stTC[:], stT8[:, 0:4], stT8[:, 4:8])
            mq = bnp.tile([P, 4], f32)
            nc.vector.tensor_scalar_mul(mq[:], stTC[:], 1.0 / N)
            varT = bnp.tile([P, KC2], f32)
            nc.vector.tensor_mul(varT[:], mq[:, 0:2], mq[:, 0:2])
            nc.vector.tensor_sub(varT[:], mq[:, 2:4], varT[:])
            epstP = bnp.tile([P, 1], f32)
            nc.vector.memset(epstP[:], EPS)
            sdT = bnp.tile([P, KC2], f32)
            nc.scalar.activation(sdT[:], varT[:], AF.Sqrt, bias=epstP[:])
            rstdT = bnp.tile([P, KC2], f32)
            nc.vector.reciprocal(rstdT[:], sdT[:])
            acT = bnp.tile([P, KC2, 2], f32)
            nc.vector.tensor_mul(acT[:, :, 0], rstdT[:], g1t_s[:])
            tmpT = bnp.tile([P, KC2], f32)
            nc.vector.tensor_mul(tmpT[:], mq[:, 0:2], acT[:, :, 0])
            nc.vector.tensor_sub(acT[:, :, 1], b1t_s[:], tmpT[:])

            # per 7-window group: batched BN+ReLU then per-window W2 matmuls
            GH = 7
            assert WPC % GH == 0
            with (
                tc.tile_pool(name="bnr", bufs=10) as bpool,
                tc.tile_pool(name="h2p", bufs=8, space="PSUM") as h2pool,
            ):
                h2ap = h2shard[0:SLOTS, :].rearrange("(t p) d -> p t d", p=P)
                for g in range(WPC // GH):
                    ws = g * GH
                    bnrs = []
                    for c in range(KC2):
                        bnr = bpool.tile([P, GH, P], f16, tag=f"bnr{c}")
                        nc.vector.tensor_scalar(
                            bnr[:], o1T[:, ws : ws + GH, c, :],
                            acT[:, c, 0:1], acT[:, c, 1:2],
                            mybir.AluOpType.mult, mybir.AluOpType.add,
                        )
                        nc.vector.tensor_scalar_max(bnr[:], bnr[:], 0.0)
                        bnrs.append(bnr)
                    for t in range(GH):
                        h2ps = h2pool.tile([P, D2], f32, tag="h2ps")
                        for c in range(KC2):
                            nc.tensor.matmul(
                                out=h2ps[:], lhsT=bnrs[c][:, t, :], rhs=w2_s[:, c, :],
                                start=(c == 0), stop=(c == KC2 - 1),
                            )
                        if (ws + t) % 4 == 3:
                            nc.vector.tensor_scalar_mul(
                                h2stage[:, ws + t, :], h2ps[:],
                                dismy_s[:, 1, ws + t : ws + t + 1],
                            )
                        else:
                            nc.scalar.activation(
                                h2stage[:, ws + t, :], h2ps[:], AF.Copy,
                                scale=dismy_s[:, 1, ws + t : ws + t + 1],
                            )
                    nc.sync.dma_start(
                        out=h2ap[:, ws : ws + GH, :],
                        in_=h2stage[:, ws : ws + GH, :],
                    )
            nc.gpsimd.collective_compute(
                "AllGather", mybir.AluOpType.bypass,
                ins=[h2shard[:, :]], outs=[h2tab[:, :]], replica_groups=RG,
            )
            if phases < 4:
                outst = cpool.tile([P, WPC, D2], f32)
                nc.vector.tensor_copy(out=outst[:], in_=h2stage[:])
                nc.gpsimd.dma_start(
                    out=out_d[0:SLOTS, :].rearrange("(t p) d -> p t d", p=P),
                    in_=outst[:],
                )
                return nc

            # ---------------- conv2 ----------------
            es1.close()  # free conv1 SBUF
            o2_pool = es.enter_context(tc.tile_pool(name="o2", bufs=1))
            o2_all = o2_pool.tile([P, WPC, D2], f16)
            lo2 = h2tab[0:HB, :]
            hi2 = h2tab[HB:NROWS, :]
            with (
                tc.tile_pool(name="g2", bufs=5) as gpool2,
                tc.tile_pool(name="sq2", bufs=6) as sqpool2,
                tc.tile_pool(name="c2p", bufs=4, space="PSUM") as wpool2,
                tc.tile_pool(name="st2p", bufs=1, space="PSUM") as stpool2,
            ):
                st2A_s = stpool2.tile([1, D2], f32, tag="st2A_s")
                st2A_q = stpool2.tile([1, D2], f32, tag="st2A_q")
                st2B_s = stpool2.tile([1, D2], f32, tag="st2B_s")
                st2B_q = stpool2.tile([1, D2], f32, tag="st2B_q")
                SPLITW = max(1, WPC - 7)
                for w in range(WPC):
                    nl, nh = int(NL[w]), int(NH[w])
                    nt = nl + nh
                    gb = gpool2.tile([P, NTmax, D2], f16, tag="g2")
                    pieces = 3 if w == WPC - 1 else 2 if (w == WPC - 2 or w == 0) else 1
                    for n0, bs, view, off in ((nl, 0, lo2, offL[w]),
                                              (nh, nl, hi2, offH[w])):
                        if not n0:
                            continue
                        cuts = [n0 * i // pieces for i in range(pieces + 1)]
                        for a, b in zip(cuts[:-1], cuts[1:]):
                            if b == a:
                                continue
                            nc.gpsimd.dma_gather(
                                gb[:, bs + a : bs + b, :], view,
                                idx_s[:, off // 16 + a * 8 : off // 16 + b * 8],
                                (b - a) * P, (b - a) * P, D2,
                                elem_step=D2, single_packet=False,
                            )
                    ps = wpool2.tile([P, D2], f32, tag="win2")
                    # self-loop: h2stage row w IS dis*h2 for my slots
                    nc.tensor.matmul(
                        out=ps[:], lhsT=ident_b[:], rhs=h2stage[:, w, :],
                        start=True, stop=(nt == 0),
                    )
                    for j in range(nt):
                        nc.tensor.matmul(
                            out=ps[:], lhsT=ident_b[:], rhs=gb[:, j, :],
                            start=False, stop=(j == nt - 1),
                        )
                    nc.vector.tensor_scalar_mul(
                        o2_all[:, w, :], ps[:], dismy_s[:, 1, w : w + 1]
                    )
                    sq = sqpool2.tile([P, D2], f16, tag="sq2")
                    nc.vector.tensor_mul(sq[:], o2_all[:, w, :], o2_all[:, w, :])
                    mcol = mask_b[:, 1:2] if w == WPC - 1 else mask_b[:, 0:1]
                    sp_s, sp_q = (st2A_s, st2A_q) if w < SPLITW else (st2B_s, st2B_q)
                    w0, w1 = (0, SPLITW - 1) if w < SPLITW else (SPLITW, WPC - 1)
                    nc.tensor.matmul(
                        out=sp_s[:], lhsT=mcol, rhs=o2_all[:, w, :],
                        start=(w == w0), stop=(w == w1), skip_group_check=True,
                    )
                    nc.tensor.matmul(
                        out=sp_q[:], lhsT=mcol, rhs=sq[:],
                        start=(w == w0), stop=(w == w1), skip_group_check=True,
                    )
                    if w == SPLITW - 1:
                        # ship the w0..SPLITW-1 partial sums early so the BN2
                        # stats AllGather fires right at conv2 end
                        stats2A = o2_pool.tile([1, 2 * D2], f32)
                        nc.vector.tensor_copy(out=stats2A[:, 0:D2], in_=st2A_s[:])
                        nc.vector.tensor_copy(out=stats2A[:, D2:], in_=st2A_q[:])
                        nc.scalar.dma_start(out=ar2_in[:, 0 : 2 * D2], in_=stats2A[:])
                stats2B = o2_pool.tile([1, 2 * D2], f32)
                nc.vector.tensor_copy(out=stats2B[:, 0:D2], in_=st2B_s[:])
                nc.vector.tensor_copy(out=stats2B[:, D2:], in_=st2B_q[:])
                nc.scalar.dma_start(out=ar2_in[:, 2 * D2 :], in_=stats2B[:])
            if phases < 5:
                outst = cpool.tile([P, WPC, D2], f32)
                nc.vector.tensor_copy(out=outst[:], in_=o2_all[:, :, :])
                nc.gpsimd.dma_start(
                    out=out_d[0:SLOTS, :].rearrange("(t p) d -> p t d", p=P),
                    in_=outst[:],
                )
                return nc
            nc.gpsimd.collective_compute(
                "AllGather", mybir.AluOpType.bypass,
                ins=[ar2_in[:, :]], outs=[ar2_out[:, :]], replica_groups=RG,
            )

            # ---------------- BN2 + output ----------------
            sg28 = o2_pool.tile([N_CORES, 4 * D2], f32)
            nc.sync.dma_start(out=sg28[:], in_=ar2_out[:, :])
            ones82 = o2_pool.tile([N_CORES, 1], f32)
            nc.vector.memset(ones82[:], 1.0)
            mq2 = o2_pool.tile([1, 2 * D2], f32)
            with tc.tile_pool(name="sg2p", bufs=1, space="PSUM") as sg2pool:
                sg2ps = sg2pool.tile([1, 4 * D2], f32, tag="sg2ps")
                nc.tensor.matmul(out=sg2ps[:], lhsT=ones82[:], rhs=sg28[:], start=True, stop=True)
                sg2c = o2_pool.tile([1, 2 * D2], f32)
                nc.vector.tensor_copy(out=sg2c[:], in_=sg2ps[:, 0 : 2 * D2])
                nc.vector.tensor_add(sg2c[:], sg2c[:], sg2ps[:, 2 * D2 :])
                nc.vector.tensor_scalar_mul(mq2[:], sg2c[:], 1.0 / N)
            mean2 = mq2[:, 0:D2]
            ex22 = mq2[:, D2:]
            var2 = o2_pool.tile([1, D2], f32)
            nc.vector.tensor_mul(var2[:], mean2, mean2)
            nc.vector.tensor_sub(var2[:], ex22, var2[:])
            epst2 = o2_pool.tile([1, 1], f32)
            nc.vector.memset(epst2[:], EPS)
            sd2 = o2_pool.tile([1, D2], f32)
            nc.scalar.activation(sd2[:], var2[:], AF.Sqrt, bias=epst2[:])
            rstd2 = o2_pool.tile([1, D2], f32)
            nc.vector.reciprocal(rstd2[:], sd2[:])
            a2 = o2_pool.tile([1, D2], f32)
            nc.vector.tensor_mul(a2[:], rstd2[:], gb2_s[:, 0:D2])
            c2 = o2_pool.tile([1, D2], f32)
            nc.vector.tensor_mul(c2[:], mean2, a2[:])
            nc.vector.tensor_sub(c2[:], gb2_s[:, D2:], c2[:])

            # broadcast a2/c2 across partitions, tiled GW-wide
            GW = 7
            assert WPC % GW == 0
            onesrow = o2_pool.tile([1, P], f16)
            nc.vector.memset(onesrow[:], 1.0)
            ac2row = o2_pool.tile([1, 2, GW, D2], f16)
            for b, srct in ((0, a2), (1, c2)):
                nc.vector.tensor_copy(out=ac2row[:, b, 0, :], in_=srct[:])
                done = 1
                while done < GW:
                    n = min(done, GW - done)
                    nc.vector.tensor_copy(
                        out=ac2row[:, b, done : done + n, :],
                        in_=ac2row[:, b, 0:n, :],
                    )
                    done += n
            acb = o2_pool.tile([P, 2, GW, D2], f16)
            acb_flat = acb[:].rearrange("p b c d -> p (b c d)")
            ac2_flat = ac2row[:].rearrange("a b c d -> a (b c d)")
            CHK = 448  # <= 512 f32 per PSUM bank
            with tc.tile_pool(name="bn2p", bufs=4, space="PSUM") as bn2p:
                for i in range(0, 2 * GW * D2, CHK):
                    bps = bn2p.tile([P, CHK], f32, tag="b2a")
                    nc.tensor.matmul(out=bps[:], lhsT=onesrow[:],
                                     rhs=ac2_flat[:, i : i + CHK],
                                     start=True, stop=True)
                    nc.scalar.activation(acb_flat[:, i : i + CHK], bps[:], AF.Copy)

            outst = o2_pool.tile([P, WPC, D2], f16)
            outap = out_d[0:SLOTS, :].rearrange("(t p) d -> p t d", p=P)
            with tc.tile_pool(name="of32", bufs=8) as ofpool:
                ngroups = WPC // GW
                spans = [(g * GW, GW) for g in range(ngroups - 1)]
                spans += [((ngroups - 1) * GW + t, 1) for t in range(GW)]
                for s0, n in spans:
                    sl = slice(s0, s0 + n)
                    nc.vector.tensor_mul(outst[:, sl, :], o2_all[:, sl, :], acb[:, 0, 0:n])
                    nc.vector.tensor_add(outst[:, sl, :], outst[:, sl, :], acb[:, 1, 0:n])
                    of32 = ofpool.tile([P, GW, D2], f32, tag="of32")
                    nc.scalar.activation(of32[:, 0:n], outst[:, sl, :], AF.Copy)
                    nc.sync.dma_start(out=outap[:, sl, :], in_=of32[:, 0:n])

    return nc


# ---------------------------------------------------------------- entry point

def _run(x, edge_index, W1, gamma1, beta1, W2, gamma2, beta2, cfg, trace=False):
    from concourse.bass_utils import run_bass_kernel_spmd

    N = cfg["N"]
    pp = _preprocess(edge_index, N)
    xtab, xown, dismy2, w1b, w2b, g1t, b1t, i2 = _pack_inputs(
        np.asarray(x, np.float32), np.asarray(W1, np.float32),
        np.asarray(W2, np.float32), gamma1, beta1, pp, cfg)
    nc = _build_kernel(cfg, pp, phases=int(__import__("os").environ.get("K_PHASES", "5")))
    nc.compile()

    shared = {
        "xtab": np.ascontiguousarray(xtab),
        "i2": np.ascontiguousarray(i2),
        "w1b": np.ascontiguousarray(w1b),
        "w2b": np.ascontiguousarray(w2b),
        "g1t": np.ascontiguousarray(g1t),
        "b1t": np.ascontiguousarray(b1t),
        "statmask": np.ascontiguousarray(pp["statmask"]),
        "gamma2": np.asarray(gamma2, np.float32).reshape(1, -1),
        "beta2": np.asarray(beta2, np.float32).reshape(1, -1),
    }
    in_maps = []
    for c in range(N_CORES):
        m = dict(shared)
        m["idx"] = np.ascontiguousarray(pp["idx_wrapped"][c])
        m["xown"] = np.ascontiguousarray(xown[c])
        m["dismy"] = np.ascontiguousarray(dismy2[c])
        in_maps.append(m)

    res = run_bass_kernel_spmd(nc, in_maps, core_ids=list(range(N_CORES)), trace=trace)
    _run.last_nc = nc

    D2 = cfg["D2"]
    out = np.empty((N, D2), np.float32)
    pos, core_of = pp["pos"], pp["core_of"]
    for c in range(N_CORES):
        nodes = np.flatnonzero(core_of == c)
        out[nodes] = res.results[c]["out"][pos[nodes]]
    _run.last_result = res
    return out


def kernel(x, edge_index, W1, b1, gamma1, beta1, W2, b2, gamma2, beta2):
    # b1/b2 cancel exactly through BatchNorm's mean subtraction; unused.
    return _run(x, edge_index, W1, gamma1, beta1, W2, gamma2, beta2, _FULL_CFG)


# revision 28
# speedup vs baseline: 1.0221x; 1.0221x over previous
"""GCN encoder (2x GCNConv + BatchNorm + ReLU) on 8 Trainium2 NeuronCores.

Strategy (graph/data parallel, per sharding hint):
- Nodes are permuted (degree-sorted, round-robin dealt) and sharded across the
  8 cores; each core owns 49 "windows" of 128 destination nodes.
- conv1 aggregates FIRST, transforms SECOND: out1 = ((D^-.5 A D^-.5) x) @ W1.
  The gathered table is dis*x quantized to fp8 e3m4 (512 B rows, so the DMA
  cost model's <512B 2x latency multiplier doesn't apply), pre-scaled by a
  power-of-two s for fp8 range; s is divided back out via the dis[d]
  diagonal.  There is no per-core x@W1 phase at all - the host ships the fp8
  table directly, saving ~215 us of xb-load/h1-write DMA and ~170 us of
  redundant PE time per core versus materializing h1 = x@W1 on device.
- Aggregation happens TRANSPOSED: for each gathered chunk (128 messages) and
  each 128-feature group, one matmul chunk_fg^T @ diag(dis[d]/s) accumulates
  into feature-major PSUM.  This lands the window aggregate directly in the
  layout BatchNorm/W1 need (no transpose stage, no slot-major eviction) and
  folds the dis[d] scaling in for free.  The per-bank pairs of accumulation
  groups rely on PSUM pending-zero semantics (start=True only on the first
  group member per bank).
- Windows are processed in an interleaved light/heavy order (they are
  degree-sorted, so the head windows have ~65 chunks and the tail ~20): a
  PE-bound heavy window always has a DMA-bound light neighbor in flight.
  The gather index stream is laid out in this processing order; the last
  processed windows are light so the pipeline drains quickly.
- o1T = W1^T @ aggT is computed one window behind the aggregation (software
  pipelining); BN1 statistics are free-dim DVE reductions of o1T, with the
  partial sums over the first 42 processed windows shipped early so the
  cross-core stats AllGather ([128,8] + on-chip sum) fires right at conv1
  end.  BN2 stats are split the same way (PSUM groups A/B).
- Self loops never enter the gather stream: conv1's self message is the
  core's own fp8 table row (xown, resident in SBUF), added as one extra
  matmul per window; conv2's is the core's own h2stage row.
- conv1 and conv2 share ONE int16 index stream: both tables use the same
  [8 x (SLOTS+1)] row layout (trailing zero row per core block) and both
  convs aggregate the same real-edge set.  The stream is split in lo/hi
  table halves for int16 range.
- h2 = relu(bn1(o1)) @ W2 (fp16) is computed per-shard and replicated via
  AllGather; conv2 then gathers 256 B fp16 rows and aggregates slot-major
  with identity matmuls (fp8 does not help conv2: dma_gather's elem size
  floor is 256 B and sub-512 B rows pay the 2x multiplier either way).
- b1/b2 are ignored: a per-feature constant added before BatchNorm cancels
  exactly in (x - mean).
"""

import sys

sys.path.insert(0, "/opt/trn_rl_repo")

import numpy as np

N_CORES = 8
P = 128
EPS = 1e-5

_FULL_CFG = dict(N=50000, IN=512, D1=256, D2=128)


# ---------------------------------------------------------------- host preprocessing

def _preprocess(edge_index, N):
    """Graph preprocessing: node permutation, slot/chunk assignment, gather
    indices.  Pure integer work on the host."""
    src = np.asarray(edge_index[0], dtype=np.int64)
    dst = np.asarray(edge_index[1], dtype=np.int64)
    loop = np.arange(N, dtype=np.int64)
    D_all = np.concatenate([dst, loop])

    deg = np.bincount(D_all, minlength=N)  # >= 1 (self loop)
    dis = (1.0 / np.sqrt(deg.astype(np.float64))).astype(np.float32)

    real_pc = N // N_CORES
    WPC = (real_pc + P - 1) // P          # windows per core
    SLOTS = WPC * P                        # slot positions per core
    BLK = SLOTS + 1                        # +1 trailing zero row per core block

    # deal nodes to cores round-robin in degree-desc order
    order = np.argsort(-deg, kind="stable")
    core_of = np.empty(N, dtype=np.int64)
    core_of[order] = np.arange(N) % N_CORES

    half_node = core_of >= (N_CORES // 2)  # False = lo table half
    halfE = half_node[src]                 # real edges only

    deg_lo = np.bincount(dst[~halfE], minlength=N)
    deg_hi = np.bincount(dst[halfE], minlength=N)

    # position of each node within its core: sort by (deg_lo desc, deg_hi
    # desc), then re-sort blocks by deg_hi - keeps per-window max(deg_lo) and
    # max(deg_hi) both tight
    pos = np.empty(N, dtype=np.int64)
    node_by_cp = np.full((N_CORES, SLOTS), -1, dtype=np.int64)
    RESORT_BLK = 896
    for c in range(N_CORES):
        nodes_c = np.flatnonzero(core_of == c)
        o = np.lexsort((-deg_hi[nodes_c], -deg_lo[nodes_c]))
        for s in range(0, len(o), RESORT_BLK):
            blk = o[s : s + RESORT_BLK]
            o[s : s + RESORT_BLK] = blk[np.argsort(-deg_hi[nodes_c][blk], kind="stable")]
        snodes = nodes_c[o]
        pos[snodes] = np.arange(len(snodes))
        node_by_cp[c, : len(snodes)] = snodes

    # per-core per-window chunk counts -> global max (SPMD static shapes)
    dlo_cp = np.zeros((N_CORES, SLOTS), dtype=np.int64)
    dhi_cp = np.zeros((N_CORES, SLOTS), dtype=np.int64)
    m = node_by_cp >= 0
    dlo_cp[m] = deg_lo[node_by_cp[m]]
    dhi_cp[m] = deg_hi[node_by_cp[m]]
    NL = dlo_cp.reshape(N_CORES, WPC, P).max(axis=2).max(axis=0)
    NH = dhi_cp.reshape(N_CORES, WPC, P).max(axis=2).max(axis=0)

    # window processing order: interleave light (tail) and heavy (head)
    # windows so DMA-bound and PE-bound windows overlap in the pipeline
    worder = []
    lo_i, hi_i = WPC - 1, 0
    while lo_i >= hi_i:
        worder.append(lo_i)
        if hi_i < lo_i:
            worder.append(hi_i)
        lo_i -= 1
        hi_i += 1
    # drain the pipeline on light windows: move the 2nd/3rd-lightest (ids
    # WPC-2, WPC-3) to the end of the processing order
    if WPC > 6:
        for wl in (WPC - 3, WPC - 2):
            worder.remove(wl)
            worder.append(wl)
    worder = np.array(worder)

    # gather index stream: segments laid out in processing order, each
    # window's segment is [lo seg][hi seg], chunk-major
    seg = (NL + NH) * P
    base = np.concatenate([[0], np.cumsum(seg[worder])])
    offL = np.empty(WPC, np.int64)
    offL[worder] = base[:-1]
    offH = offL + NL * P
    TOT = int(base[-1])

    # chunk index of each edge among its (dst, half) group
    key = dst * 2 + halfE
    ksort = np.argsort(key, kind="stable")
    skey = key[ksort]
    starts = np.concatenate([[0], np.flatnonzero(np.diff(skey)) + 1])
    group_len = np.diff(np.concatenate([starts, [len(skey)]]))
    chunk_sorted = np.arange(len(skey)) - np.repeat(starts, group_len)
    chunk = np.empty(len(src), dtype=np.int64)
    chunk[ksort] = chunk_sorted

    cD = core_of[dst]
    wD = pos[dst] // P
    slotD = pos[dst] % P
    absrow = core_of[src] * BLK + pos[src]
    rel = np.where(~halfE, absrow, absrow - (N_CORES // 2) * BLK)
    assert rel.max() < 32768
    epos = np.where(~halfE, offL[wD], offH[wD]) + chunk * P + slotD

    PADIDX = SLOTS  # block 0's trailing zero row (rel within half view)
    flat = np.full(N_CORES * TOT, PADIDX, dtype=np.int16)
    flat[cD * TOT + epos] = rel.astype(np.int16)
    flat = flat.reshape(N_CORES, TOT)
    # wrap: idx i -> [i%16, i//16], replicated across the 8 groups of 16
    wrapped16 = flat.reshape(N_CORES, TOT // 16, 16).transpose(0, 2, 1)
    idx_wrapped = np.tile(wrapped16, (1, P // 16, 1))

    # per-core dis (by slot), 1.0 for dummies
    dis_cp = np.ones((N_CORES, SLOTS), dtype=np.float32)
    dis_cp[m] = dis[node_by_cp[m]]
    dismy = dis_cp.reshape(N_CORES, WPC, P).transpose(0, 2, 1)  # [c, 128, WPC]

    # stats mask for conv2 (dummy slots have nonzero h2 = relu(bn1(0)))
    n_dummy = SLOTS - real_pc
    statmask = np.ones((P, 2), dtype=np.float32)
    if n_dummy:
        statmask[P - n_dummy :, 1] = 0.0

    waste = float(TOT) / max(1, len(src) / N_CORES) - 1.0
    return dict(
        WPC=WPC, SLOTS=SLOTS, BLK=BLK, worder=worder,
        NL=NL.astype(int), NH=NH.astype(int), TOT=TOT,
        offL=offL, offH=offH, idx_wrapped=idx_wrapped,
        dismy=dismy, statmask=statmask,
        node_by_cp=node_by_cp, pos=pos, core_of=core_of,
        dis=dis, real_pc=real_pc, waste=waste,
    )


def _pack_inputs(x, W1, W2, gamma1, beta1, pp, cfg):
    """Build the per-core / shared device input arrays."""
    import ml_dtypes

    f16 = np.float16
    f8 = ml_dtypes.float8_e3m4
    N, IN, D1, D2 = cfg["N"], cfg["IN"], cfg["D1"], cfg["D2"]
    WPC, SLOTS, BLK = pp["WPC"], pp["SLOTS"], pp["BLK"]
    KC, KC2 = IN // P, D1 // P
    NROWS = N_CORES * BLK

    # fp8 quantization scale: power of two, headroom below e3m4 max (15.5),
    # large enough to keep the bulk of values out of the denormal zone
    v = np.asarray(x, np.float32) * pp["dis"][:, None]
    maxv = float(np.abs(v).max())
    s = float(2.0 ** np.floor(np.log2(14.0 / maxv))) if maxv > 0 else 1.0

    xtab = np.zeros((NROWS, IN), dtype=f8)
    m = pp["node_by_cp"] >= 0
    rows = (np.arange(N_CORES)[:, None] * BLK + np.arange(SLOTS)[None, :])[m]
    xtab[rows] = (v[pp["node_by_cp"][m]] * s).astype(f8)

    xown = np.empty((N_CORES, P, WPC, IN), dtype=f8)
    for c in range(N_CORES):
        xown[c] = xtab[c * BLK : c * BLK + SLOTS].reshape(WPC, P, IN).transpose(1, 0, 2)

    # dismy2[c, :, 0, :] = dis/s (conv1 eviction), [:, 1, :] = dis (h2 / conv2)
    dismy2 = np.empty((N_CORES, P, 2, WPC), dtype=np.float32)
    dismy2[:, :, 0, :] = pp["dismy"] / s
    dismy2[:, :, 1, :] = pp["dismy"]

    w1b = W1.reshape(KC, P, D1).transpose(1, 0, 2).astype(f16)        # [p, kc, D1]
    w2b = W2.reshape(KC2, P, D2).transpose(1, 0, 2).astype(f16)       # [p, kc, D2]
    g1t = np.asarray(gamma1, np.float32).reshape(KC2, P).T.copy()     # [p, kc2]
    b1t = np.asarray(beta1, np.float32).reshape(KC2, P).T.copy()

    i2 = np.zeros((P, P), dtype=f8)
    i2[np.arange(P), np.arange(P)] = 1.0
    return xtab, xown, dismy2, w1b, w2b, g1t, b1t, i2


# ---------------------------------------------------------------- device kernel

def _build_kernel(cfg, pp, phases=5):
    import concourse.bacc as bacc
    import concourse.mybir as mybir
    import concourse.tile as tile
    from concourse.masks import make_identity
    from contextlib import ExitStack

    N, IN, D1, D2 = cfg["N"], cfg["IN"], cfg["D1"], cfg["D2"]
    WPC, SLOTS, BLK = pp["WPC"], pp["SLOTS"], pp["BLK"]
    NL, NH, TOT = pp["NL"], pp["NH"], pp["TOT"]
    offL, offH = pp["offL"], pp["offH"]
    KC = IN // P
    KC2 = D1 // P
    HB = (N_CORES // 2) * BLK        # hi half base row
    NROWS = N_CORES * BLK            # table rows
    NTmax = int((NL + NH).max())
    RG = [list(range(N_CORES))]
    f32, f16, i16 = mybir.dt.float32, mybir.dt.float16, mybir.dt.int16
    f8 = mybir.dt.float8e3
    AF = mybir.ActivationFunctionType
    AX = mybir.AxisListType
    ALU = mybir.AluOpType

    nc = bacc.Bacc(num_devices=N_CORES)

    # ---- I/O
    xtab_d = nc.dram_tensor("xtab", [NROWS, IN], f8, kind="ExternalInput")
    xown_d = nc.dram_tensor("xown", [P, WPC, IN], f8, kind="ExternalInput")
    idx_d = nc.dram_tensor("idx", [P, TOT // 16], i16, kind="ExternalInput")
    dismy_d = nc.dram_tensor("dismy", [P, 2, WPC], f32, kind="ExternalInput")
    i2_d = nc.dram_tensor("i2", [P, P], f8, kind="ExternalInput")
    w1_d = nc.dram_tensor("w1b", [P, KC, D1], f16, kind="ExternalInput")
    w2_d = nc.dram_tensor("w2b", [P, KC2, D2], f16, kind="ExternalInput")
    g1t_d = nc.dram_tensor("g1t", [P, KC2], f32, kind="ExternalInput")
    b1t_d = nc.dram_tensor("b1t", [P, KC2], f32, kind="ExternalInput")
    mask_d = nc.dram_tensor("statmask", [P, 2], f32, kind="ExternalInput")
    g2_d = nc.dram_tensor("gamma2", [1, D2], f32, kind="ExternalInput")
    b2_d = nc.dram_tensor("beta2", [1, D2], f32, kind="ExternalInput")
    out_d = nc.dram_tensor("out", [SLOTS, D2], f32, kind="ExternalOutput")

    # ---- internal DRAM
    h2shard = nc.dram_tensor("h2shard", [BLK, D2], f16, kind="Internal")
    h2tab = nc.dram_tensor("h2tab", [NROWS, D2], f16, kind="Internal", addr_space="Shared")
    ar1_in = nc.dram_tensor("ar1_in", [P, 8], f32, kind="Internal")
    ar1_out = nc.dram_tensor("ar1_out", [N_CORES, P, 8], f32, kind="Internal", addr_space="Shared")
    ar2_in = nc.dram_tensor("ar2_in", [1, 4 * D2], f32, kind="Internal")
    ar2_out = nc.dram_tensor("ar2_out", [N_CORES, 4 * D2], f32, kind="Internal", addr_space="Shared")

    with tile.TileContext(nc) as tc:
        es = ExitStack()
        with es:
            cpool = es.enter_context(tc.tile_pool(name="const", bufs=1))
            # idx stream first: its load DMA heads the queue, the first
            # gather depends on it
            idx_s = cpool.tile([P, TOT // 16], i16)
            wo = pp["worder"]
            c0 = int(offL[wo[6]]) // 16 if WPC > 6 else TOT // 16
            nc.sync.dma_start(out=idx_s[:, 0:c0], in_=idx_d[:, 0:c0])
            ident_b = cpool.tile([P, P], f16)
            make_identity(nc, ident_b[:])
            i2_s = cpool.tile([P, P], f8)
            nc.scalar.dma_start(out=i2_s[:], in_=i2_d[:, :])
            w1_s = cpool.tile([P, KC, D1], f16)
            nc.scalar.dma_start(out=w1_s[:], in_=w1_d[:, :, :])
            w2_s = cpool.tile([P, KC2, D2], f16)
            nc.scalar.dma_start(out=w2_s[:], in_=w2_d[:, :, :])
            dismy_s = cpool.tile([P, 2, WPC], f32)
            nc.scalar.dma_start(out=dismy_s[:], in_=dismy_d[:, :, :])
            g1t_s = cpool.tile([P, KC2], f32)
            nc.scalar.dma_start(out=g1t_s[:], in_=g1t_d[:, :])
            b1t_s = cpool.tile([P, KC2], f32)
            nc.scalar.dma_start(out=b1t_s[:], in_=b1t_d[:, :])
            mask_s = cpool.tile([P, 2], f32)
            nc.scalar.dma_start(out=mask_s[:], in_=mask_d[:, :])
            mask_b = cpool.tile([P, 2], f16)
            nc.vector.tensor_copy(out=mask_b[:], in_=mask_s[:])
            gb2_s = cpool.tile([1, 2 * D2], f32)  # gamma2|beta2
            nc.scalar.dma_start(out=gb2_s[:, 0:D2], in_=g2_d[:, :])
            nc.scalar.dma_start(out=gb2_s[:, D2:], in_=b2_d[:, :])
            # h2stage doubles as the conv2 self-loop rhs
            h2stage = cpool.tile([P, WPC, D2], f16)

            # zero pad row of h2shard (row SLOTS)
            zrow2 = cpool.tile([1, D2], f16)
            nc.vector.memset(zrow2[:], 0)
            nc.sync.dma_start(out=h2shard[SLOTS:BLK, :], in_=zrow2[:])

            # conv1-lifetime pools
            es1 = ExitStack()
            o1_pool = es1.enter_context(tc.tile_pool(name="o1", bufs=1))
            xown_s = o1_pool.tile([P, WPC, IN], f8)
            # first two processed windows (ids WPC-1 and 0) up front
            nc.sync.dma_start(out=xown_s[:, 0:1], in_=xown_d[:, 0:1, :])
            nc.sync.dma_start(out=xown_s[:, WPC - 1 :], in_=xown_d[:, WPC - 1 :, :])
            o1T = o1_pool.tile([P, WPC, KC2, P], f16)
            stacc = o1_pool.tile([P, WPC, 4], f32)

            # ---------------- conv1: aggregate-first windows ----------------
            lo_view = xtab_d[0:HB, :]
            hi_view = xtab_d[HB:NROWS, :]
            KD, KP = int(__import__('os').environ.get('K_KD', 999)), int(__import__('os').environ.get('K_KP', 0))          # tail chunks summed on DVE / Pool per window
            SPLITW = max(1, WPC - 7)   # pipeline the stats reduction at this window
            with (
                tc.tile_pool(name="g1", bufs=3) as gpool,
                tc.tile_pool(name="ag", bufs=4) as apool,
                tc.tile_pool(name="at", bufs=3) as atpool,
                tc.tile_pool(name="sq1", bufs=4) as sqpool,
                tc.tile_pool(name="acd", bufs=3) as adpool,
                tc.tile_pool(name="c1p", bufs=3, space="PSUM") as wpool,
                tc.tile_pool(name="c1q", bufs=3, space="PSUM") as wpool2c,
                tc.tile_pool(name="o1p", bufs=2, space="PSUM") as o1ppool,
            ):
                aggq, trq = [], []
                # interleaved light/heavy processing order (see _preprocess):
                # a heavy PE-bound window always has a light DMA-bound
                # neighbor in flight, so neither engine starves
                for k, w in enumerate(pp["worder"]):
                    nl, nh = int(NL[w]), int(NH[w])
                    nt = nl + nh
                    gb = gpool.tile([P, NTmax, IN], f8, tag="g1")
                    pieces = 3 if k == 0 else 2 if k <= 2 or k >= WPC - 2 else 1
                    for n0, bs, view, off in ((nl, 0, lo_view, offL[w]),
                                              (nh, nl, hi_view, offH[w])):
                        if not n0:
                            continue
                        cuts = [n0 * i // pieces for i in range(pieces + 1)]
                        for a, b in zip(cuts[:-1], cuts[1:]):
                            if b == a:
                                continue
                            nc.gpsimd.dma_gather(
                                gb[:, bs + a : bs + b, :], view,
                                idx_s[:, off // 16 + a * 8 : off // 16 + b * 8],
                                (b - a) * P, (b - a) * P, IN,
                                elem_step=IN, single_packet=False,
                            )
                    if k == 1:
                        # deferred bulk loads: issued from the Pool stream here
                        # so the first windows' gather transfers aren't queued
                        # behind them on the DMA engines
                        if c0 < TOT // 16:
                            nc.gpsimd.dma_start(out=idx_s[:, c0:], in_=idx_d[:, c0:])
                        nc.gpsimd.dma_start(out=xown_s[:, 1 : WPC - 1], in_=xown_d[:, 1 : WPC - 1, :])
                    assert nt > 0
                    # heavy windows are PE-bound: pre-sum their tail chunks on
                    # the (otherwise idle) DVE into one f16 partial, which then
                    # joins the aggregation as a single extra chunk
                    kd = max(0, min(12, nt - 32))
                    if kd >= 2:
                        accD = adpool.tile([P, IN], f16, tag="accD")
                        nc.vector.tensor_add(accD[:], gb[:, nt - kd, :], gb[:, nt - kd + 1, :])
                        for j in range(nt - kd + 2, nt):
                            nc.vector.tensor_add(accD[:], accD[:], gb[:, j, :])
                    else:
                        kd = 0
                    # diag(dis[d]/s) for this window, used as the moving rhs
                    # of every aggregation matmul: out += chunk_fg^T @ diag
                    # lands the aggregate directly FEATURE-MAJOR and
                    # dis-scaled - no transpose stage, no slot-major evict
                    diag = apool.tile([P, P], f16, tag="diag")
                    nc.vector.tensor_scalar_mul(
                        diag[:], ident_b[:], dismy_s[:, 0, w : w + 1]
                    )
                    # two PSUM tiles of two feature-chunk accumulators each;
                    # the 4 groups per bank-pair rely on pending-zero: only
                    # the first matmul of each bank uses start=True
                    psA = wpool.tile([P, 2, P], f32, tag="psA")
                    psB = wpool2c.tile([P, 2, P], f32, tag="psB")
                    srcs = [gb[:, j, :] for j in range(nt - kd)]
                    srcs.append(xown_s[:, w, :])
                    if kd:
                        srcs.append(accD[:])
                    for j, src_ap in enumerate(srcs):
                        for fg in range(KC):
                            tgt, fh = (psA, fg) if fg < 2 else (psB, fg - 2)
                            nc.tensor.matmul(
                                out=tgt[:, fh, :],
                                lhsT=src_ap[:, fg * P : (fg + 1) * P],
                                rhs=diag[:],
                                start=(j == 0 and fh == 0),
                                stop=(j == len(srcs) - 1 and fh == 1),
                                skip_group_check=True,
                            )
                    aggq.append((w, k, psA, psB))

                    def do_w1(wd, kd2, pA, pB):
                        # evict the transposed aggregate, then one PSUM group
                        # for both W1 halves
                        aT = atpool.tile([P, KC, P], f16, tag="at")
                        nc.scalar.activation(aT[:, 0:2, :], pA[:], AF.Copy)
                        nc.scalar.activation(aT[:, 2:4, :], pB[:], AF.Copy)
                        trq.append((wd, kd2, aT))

                    def do_w1_mm(wd, kd2, aT):
                        op = o1ppool.tile([P, KC2, P], f32, tag="o1p")
                        for h in range(KC2):
                            for kc in range(KC):
                                nc.tensor.matmul(
                                    out=op[:, h, :],
                                    lhsT=w1_s[:, kc, h * P : (h + 1) * P],
                                    rhs=aT[:, kc, :],
                                    start=(h == 0 and kc == 0),
                                    stop=(h == KC2 - 1 and kc == KC - 1),
                                    skip_group_check=True,
                                )
                        nc.scalar.activation(o1T[:, wd, :, :], op[:], AF.Copy)
                        sq = sqpool.tile([P, KC2, P], f16, tag="sq")
                        nc.vector.tensor_mul(sq[:], o1T[:, wd], o1T[:, wd])
                        nc.vector.tensor_reduce(
                            stacc[:, kd2, 0:2], o1T[:, wd], axis=AX.X, op=ALU.add
                        )
                        nc.vector.tensor_reduce(
                            stacc[:, kd2, 2:4], sq[:], axis=AX.X, op=ALU.add
                        )
                        if kd2 == SPLITW - 1:
                            # exact partial sums over the first SPLITW processed
                            # windows, shipped early so the stats AllGather
                            # fires right at conv1 end
                            stA = o1_pool.tile([P, 4], f32)
                            nc.vector.tensor_reduce(
                                stA[:], stacc[:, 0:SPLITW].rearrange("p w f -> p f w"),
                                axis=AX.X, op=ALU.add,
                            )
                            nc.scalar.dma_start(out=ar1_in[:, 0:4], in_=stA[:])

                    def pipeline_step(last):
                        if len(aggq) > (0 if last else 1):
                            do_w1(*aggq.pop(0))
                        if len(trq) > (0 if last else 1):
                            do_w1_mm(*trq.pop(0))

                    pipeline_step(False)
                    if k == WPC - 1:
                        while aggq or trq:
                            pipeline_step(True)
                stB = o1_pool.tile([P, 4], f32)
                nc.vector.tensor_reduce(
                    stB[:], stacc[:, SPLITW:].rearrange("p w f -> p f w"),
                    axis=AX.X, op=ALU.add,
                )
                nc.scalar.dma_start(out=ar1_in[:, 4:8], in_=stB[:])
            if phases < 3:
                outst = cpool.tile([P, WPC, D2], f32)
                nc.vector.memset(outst[:], 0)
                nc.gpsimd.dma_start(
                    out=out_d[0:SLOTS, :].rearrange("(t p) d -> p t d", p=P),
                    in_=outst[:],
                )
                return nc
            nc.gpsimd.collective_compute(
                "AllGather", mybir.AluOpType.bypass,
                ins=[ar1_in[:, :]], outs=[ar1_out[:, :, :]], replica_groups=RG,
            )

            # ---------------- BN1 factors + h2 shard ----------------
            bnp = es1.enter_context(tc.tile_pool(name="bn1", bufs=1))
            sg8 = bnp.tile([P, N_CORES, 8], f32)
            nc.sync.dma_start(out=sg8[:], in_=ar1_out[:, :, :].rearrange("c p f -> p c f"))
            stT8 = bnp.tile([P, 8], f32)
            nc.vector.tensor_reduce(
                stT8[:], sg8[:].rearrange("p c f -> p f c"), axis=AX.X, op=ALU.add
            )
            stTC = bnp.tile([P, 4], f32)
            nc.vector.tensor_add(stTC[:], stT8[:, 0:4], stT8[:, 4:8])
            mq = bnp.tile([P, 4], f32)
            nc.vector.tensor_scalar_mul(mq[:], stTC[:], 1.0 / N)
            varT = bnp.tile([P, KC2], f32)
            nc.vector.tensor_mul(varT[:], mq[:, 0:2], mq[:, 0:2])
            nc.vector.tensor_sub(varT[:], mq[:, 2:4], varT[:])
            epstP = bnp.tile([P, 1], f32)
            nc.vector.memset(epstP[:], EPS)
            sdT = bnp.tile([P, KC2], f32)
            nc.scalar.activation(sdT[:], varT[:], AF.Sqrt, bias=epstP[:])
            rstdT = bnp.tile([P, KC2], f32)
            nc.vector.reciprocal(rstdT[:], sdT[:])
            acT = bnp.tile([P, KC2, 2], f32)
            nc.vector.tensor_mul(acT[:, :, 0], rstdT[:], g1t_s[:])
            tmpT = bnp.tile([P, KC2], f32)
            nc.vector.tensor_mul(tmpT[:], mq[:, 0:2], acT[:, :, 0])
            nc.vector.tensor_sub(acT[:, :, 1], b1t_s[:], tmpT[:])

            # per 7-window group: batched BN+ReLU then per-window W2 matmuls
            GH = 7
            assert WPC % GH == 0
            with (
                tc.tile_pool(name="bnr", bufs=10) as bpool,
                tc.tile_pool(name="h2p", bufs=8, space="PSUM") as h2pool,
            ):
                h2ap = h2shard[0:SLOTS, :].rearrange("(t p) d -> p t d", p=P)
                for g in range(WPC // GH):
                    ws = g * GH
                    bnrs = []
                    for c in range(KC2):
                        bnr = bpool.tile([P, GH, P], f16, tag=f"bnr{c}")
                        nc.vector.tensor_scalar(
                            bnr[:], o1T[:, ws : ws + GH, c, :],
                            acT[:, c, 0:1], acT[:, c, 1:2],
                            mybir.AluOpType.mult, mybir.AluOpType.add,
                        )
                        nc.vector.tensor_scalar_max(bnr[:], bnr[:], 0.0)
                        bnrs.append(bnr)
                    for t in range(GH):
                        h2ps = h2pool.tile([P, D2], f32, tag="h2ps")
                        for c in range(KC2):
                            nc.tensor.matmul(
                                out=h2ps[:], lhsT=bnrs[c][:, t, :], rhs=w2_s[:, c, :],
                                start=(c == 0), stop=(c == KC2 - 1),
                            )
                        if (ws + t) % 4 == 3:
                            nc.vector.tensor_scalar_mul(
                                h2stage[:, ws + t, :], h2ps[:],
                                dismy_s[:, 1, ws + t : ws + t + 1],
                            )
                        else:
                            nc.scalar.activation(
                                h2stage[:, ws + t, :], h2ps[:], AF.Copy,
                                scale=dismy_s[:, 1, ws + t : ws + t + 1],
                            )
                    nc.sync.dma_start(
                        out=h2ap[:, ws : ws + GH, :],
                        in_=h2stage[:, ws : ws + GH, :],
                    )
            nc.gpsimd.collective_compute(
                "AllGather", mybir.AluOpType.bypass,
                ins=[h2shard[:, :]], outs=[h2tab[:, :]], replica_groups=RG,
            )
            if phases < 4:
                outst = cpool.tile([P, WPC, D2], f32)
                nc.vector.tensor_copy(out=outst[:], in_=h2stage[:])
                nc.gpsimd.dma_start(
                    out=out_d[0:SLOTS, :].rearrange("(t p) d -> p t d", p=P),
                    in_=outst[:],
                )
                return nc

            # ---------------- conv2 ----------------
            es1.close()  # free conv1 SBUF
            o2_pool = es.enter_context(tc.tile_pool(name="o2", bufs=1))
            o2_all = o2_pool.tile([P, WPC, D2], f16)
            lo2 = h2tab[0:HB, :]
            hi2 = h2tab[HB:NROWS, :]
            with (
                tc.tile_pool(name="g2", bufs=5) as gpool2,
                tc.tile_pool(name="sq2", bufs=6) as sqpool2,
                tc.tile_pool(name="c2p", bufs=4, space="PSUM") as wpool2,
                tc.tile_pool(name="st2p", bufs=1, space="PSUM") as stpool2,
            ):
                st2A_s = stpool2.tile([1, D2], f32, tag="st2A_s")
                st2A_q = stpool2.tile([1, D2], f32, tag="st2A_q")
                st2B_s = stpool2.tile([1, D2], f32, tag="st2B_s")
                st2B_q = stpool2.tile([1, D2], f32, tag="st2B_q")
                SPLITW = max(1, WPC - 7)
                for w in range(WPC):
                    nl, nh = int(NL[w]), int(NH[w])
                    nt = nl + nh
                    gb = gpool2.tile([P, NTmax, D2], f16, tag="g2")
                    pieces = 3 if w == WPC - 1 else 2 if (w == WPC - 2 or w == 0) else 1
                    for n0, bs, view, off in ((nl, 0, lo2, offL[w]),
                                              (nh, nl, hi2, offH[w])):
                        if not n0:
                            continue
                        cuts = [n0 * i // pieces for i in range(pieces + 1)]
                        for a, b in zip(cuts[:-1], cuts[1:]):
                            if b == a:
                                continue
                            nc.gpsimd.dma_gather(
                                gb[:, bs + a : bs + b, :], view,
                                idx_s[:, off // 16 + a * 8 : off // 16 + b * 8],
                                (b - a) * P, (b - a) * P, D2,
                                elem_step=D2, single_packet=False,
                            )
                    ps = wpool2.tile([P, D2], f32, tag="win2")
                    # self-loop: h2stage row w IS dis*h2 for my slots
                    nc.tensor.matmul(
                        out=ps[:], lhsT=ident_b[:], rhs=h2stage[:, w, :],
                        start=True, stop=(nt == 0),
                    )
                    for j in range(nt):
                        nc.tensor.matmul(
                            out=ps[:], lhsT=ident_b[:], rhs=gb[:, j, :],
                            start=False, stop=(j == nt - 1),
                        )
                    nc.vector.tensor_scalar_mul(
                        o2_all[:, w, :], ps[:], dismy_s[:, 1, w : w + 1]
                    )
                    sq = sqpool2.tile([P, D2], f16, tag="sq2")
                    nc.vector.tensor_mul(sq[:], o2_all[:, w, :], o2_all[:, w, :])
                    mcol = mask_b[:, 1:2] if w == WPC - 1 else mask_b[:, 0:1]
                    sp_s, sp_q = (st2A_s, st2A_q) if w < SPLITW else (st2B_s, st2B_q)
                    w0, w1 = (0, SPLITW - 1) if w < SPLITW else (SPLITW, WPC - 1)
                    nc.tensor.matmul(
                        out=sp_s[:], lhsT=mcol, rhs=o2_all[:, w, :],
                        start=(w == w0), stop=(w == w1), skip_group_check=True,
                    )
                    nc.tensor.matmul(
                        out=sp_q[:], lhsT=mcol, rhs=sq[:],
                        start=(w == w0), stop=(w == w1), skip_group_check=True,
                    )
                    if w == SPLITW - 1:
                        # ship the w0..SPLITW-1 partial sums early so the BN2
                        # stats AllGather fires right at conv2 end
                        stats2A = o2_pool.tile([1, 2 * D2], f32)
                        nc.vector.tensor_copy(out=stats2A[:, 0:D2], in_=st2A_s[:])
                        nc.vector.tensor_copy(out=stats2A[:, D2:], in_=st2A_q[:])
                        nc.scalar.dma_start(out=ar2_in[:, 0 : 2 * D2], in_=stats2A[:])
                stats2B = o2_pool.tile([1, 2 * D2], f32)
                nc.vector.tensor_copy(out=stats2B[:, 0:D2], in_=st2B_s[:])
                nc.vector.tensor_copy(out=stats2B[:, D2:], in_=st2B_q[:])
                nc.scalar.dma_start(out=ar2_in[:, 2 * D2 :], in_=stats2B[:])
            if phases < 5:
                outst = cpool.tile([P, WPC, D2], f32)
                nc.vector.tensor_copy(out=outst[:], in_=o2_all[:, :, :])
                nc.gpsimd.dma_start(
                    out=out_d[0:SLOTS, :].rearrange("(t p) d -> p t d", p=P),
                    in_=outst[:],
                )
                return nc
            nc.gpsimd.collective_compute(
                "AllGather", mybir.AluOpType.bypass,
                ins=[ar2_in[:, :]], outs=[ar2_out[:, :]], replica_groups=RG,
            )

            # ---------------- BN2 + output ----------------
            sg28 = o2_pool.tile([N_CORES, 4 * D2], f32)
            nc.sync.dma_start(out=sg28[:], in_=ar2_out[:, :])
            ones82 = o2_pool.tile([N_CORES, 1], f32)
            nc.vector.memset(ones82[:], 1.0)
            mq2 = o2_pool.tile([1, 2 * D2], f32)
            with tc.tile_pool(name="sg2p", bufs=1, space="PSUM") as sg2pool:
                sg2ps = sg2pool.tile([1, 4 * D2], f32, tag="sg2ps")
                nc.tensor.matmul(out=sg2ps[:], lhsT=ones82[:], rhs=sg28[:], start=True, stop=True)
                sg2c = o2_pool.tile([1, 2 * D2], f32)
                nc.vector.tensor_copy(out=sg2c[:], in_=sg2ps[:, 0 : 2 * D2])
                nc.vector.tensor_add(sg2c[:], sg2c[:], sg2ps[:, 2 * D2 :])
                nc.vector.tensor_scalar_mul(mq2[:], sg2c[:], 1.0 / N)
            mean2 = mq2[:, 0:D2]
            ex22 = mq2[:, D2:]
            var2 = o2_pool.tile([1, D2], f32)
            nc.vector.tensor_mul(var2[:], mean2, mean2)
            nc.vector.tensor_sub(var2[:], ex22, var2[:])
            epst2 = o2_pool.tile([1, 1], f32)
            nc.vector.memset(epst2[:], EPS)
            sd2 = o2_pool.tile([1, D2], f32)
            nc.scalar.activation(sd2[:], var2[:], AF.Sqrt, bias=epst2[:])
            rstd2 = o2_pool.tile([1, D2], f32)
            nc.vector.reciprocal(rstd2[:], sd2[:])
            a2 = o2_pool.tile([1, D2], f32)
            nc.vector.tensor_mul(a2[:], rstd2[:], gb2_s[:, 0:D2])
            c2 = o2_pool.tile([1, D2], f32)
            nc.vector.tensor_mul(c2[:], mean2, a2[:])
            nc.vector.tensor_sub(c2[:], gb2_s[:, D2:], c2[:])

            # broadcast a2/c2 across partitions, tiled GW-wide
            GW = 7
            assert WPC % GW == 0
            onesrow = o2_pool.tile([1, P], f16)
            nc.vector.memset(onesrow[:], 1.0)
            ac2row = o2_pool.tile([1, 2, GW, D2], f16)
            for b, srct in ((0, a2), (1, c2)):
                nc.vector.tensor_copy(out=ac2row[:, b, 0, :], in_=srct[:])
                done = 1
                while done < GW:
                    n = min(done, GW - done)
                    nc.vector.tensor_copy(
                        out=ac2row[:, b, done : done + n, :],
                        in_=ac2row[:, b, 0:n, :],
                    )
                    done += n
            acb = o2_pool.tile([P, 2, GW, D2], f16)
            acb_flat = acb[:].rearrange("p b c d -> p (b c d)")
            ac2_flat = ac2row[:].rearrange("a b c d -> a (b c d)")
            CHK = 448  # <= 512 f32 per PSUM bank
            with tc.tile_pool(name="bn2p", bufs=4, space="PSUM") as bn2p:
                for i in range(0, 2 * GW * D2, CHK):
                    bps = bn2p.tile([P, CHK], f32, tag="b2a")
                    nc.tensor.matmul(out=bps[:], lhsT=onesrow[:],
                                     rhs=ac2_flat[:, i : i + CHK],
                                     start=True, stop=True)
                    nc.scalar.activation(acb_flat[:, i : i + CHK], bps[:], AF.Copy)

            outst = o2_pool.tile([P, WPC, D2], f16)
            outap = out_d[0:SLOTS, :].rearrange("(t p) d -> p t d", p=P)
            with tc.tile_pool(name="of32", bufs=8) as ofpool:
                ngroups = WPC // GW
                spans = [(g * GW, GW) for g in range(ngroups - 1)]
                spans += [((ngroups - 1) * GW + t, 1) for t in range(GW)]
                for s0, n in spans:
                    sl = slice(s0, s0 + n)
                    nc.vector.tensor_mul(outst[:, sl, :], o2_all[:, sl, :], acb[:, 0, 0:n])
                    nc.vector.tensor_add(outst[:, sl, :], outst[:, sl, :], acb[:, 1, 0:n])
                    of32 = ofpool.tile([P, GW, D2], f32, tag="of32")
                    nc.scalar.activation(of32[:, 0:n], outst[:, sl, :], AF.Copy)
                    nc.sync.dma_start(out=outap[:, sl, :], in_=of32[:, 0:n])

    return nc


# ---------------------------------------------------------------- entry point

def _run(x, edge_index, W1, gamma1, beta1, W2, gamma2, beta2, cfg, trace=False):
    from concourse.bass_utils import run_bass_kernel_spmd

    N = cfg["N"]
    pp = _preprocess(edge_index, N)
    xtab, xown, dismy2, w1b, w2b, g1t, b1t, i2 = _pack_inputs(
        np.asarray(x, np.float32), np.asarray(W1, np.float32),
        np.asarray(W2, np.float32), gamma1, beta1, pp, cfg)
    nc = _build_kernel(cfg, pp, phases=int(__import__("os").environ.get("K_PHASES", "5")))
    nc.compile()

    shared = {
        "xtab": np.ascontiguousarray(xtab),
        "i2": np.ascontiguousarray(i2),
        "w1b": np.ascontiguousarray(w1b),
        "w2b": np.ascontiguousarray(w2b),
        "g1t": np.ascontiguousarray(g1t),
        "b1t": np.ascontiguousarray(b1t),
        "statmask": np.ascontiguousarray(pp["statmask"]),
        "gamma2": np.asarray(gamma2, np.float32).reshape(1, -1),
        "beta2": np.asarray(beta2, np.float32).reshape(1, -1),
    }
    in_maps = []
    for c in range(N_CORES):
        m = dict(shared)
        m["idx"] = np.ascontiguousarray(pp["idx_wrapped"][c])
        m["xown"] = np.ascontiguousarray(xown[c])
        m["dismy"] = np.ascontiguousarray(dismy2[c])
        in_maps.append(m)

    res = run_bass_kernel_spmd(nc, in_maps, core_ids=list(range(N_CORES)), trace=trace)
    _run.last_nc = nc

    D2 = cfg["D2"]
    out = np.empty((N, D2), np.float32)
    pos, core_of = pp["pos"], pp["core_of"]
    for c in range(N_CORES):
        nodes = np.flatnonzero(core_of == c)
        out[nodes] = res.results[c]["out"][pos[nodes]]
    _run.last_result = res
    return out


def kernel(x, edge_index, W1, b1, gamma1, beta1, W2, b2, gamma2, beta2):
    # b1/b2 cancel exactly through BatchNorm's mean subtraction; unused.
    return _run(x, edge_index, W1, gamma1, beta1, W2, gamma2, beta2, _FULL_CFG)
